# revision 43
# baseline (speedup 1.0000x reference)
"""nn_Attention_16965120820033 — 16-head attention with Bayesian V/proj weights.

Sharding: 8 cores = 4 batches x 2 head-groups (8 heads each), partial
sums reduced on the host (the inter-group all-reduce).

Per core, one fused flat pipeline, all bf16 matmuls (fp8/DoubleRow was
measured at 1.7-2.4e-2 max-rel-err — over budget): QKV projections,
QK^T scores (two 64-row head-halves per PSUM tile, j-grouped so one
exp ACTIVATE covers both), softmax exp on ACT (no max-subtraction —
scores are O(4)), AV with the denominator pre-broadcast via 64
ones-columns appended to V, bf16 out-projection. The PE is the
bottleneck (~327us of 512-col streams); everything else hides under it:
 - projections/out-proj ride the unit stream as PE fillers on a static
   just-in-time schedule (need_by - lead placement, CAP 2/unit);
 - each normalize's two ~4.3us DVE reciprocal chains are deferred into
   the next units' slots so filler PSUM->SBUF casts (which feed the
   next hp-block) interleave between them on the in-order DVE queue;
 - one av->sbuf copy rides the Scalar engine so the AV PSUM banks
   release without queueing behind a reciprocal;
 - out-projection is gated ~9 units past its normalize (the scheduler
   cost model underestimates reciprocals ~6x; an early outproj
   head-of-line-blocks the PE and the >3.4us idle re-throttles HAM);
 - ~20 dummy matmuls on a zeroed tile keep HAM at K=8/8 through the
   DMA-bound head; the tail normalize is split by nt-quarter with the
   out-projection inlined per quarter;
 - inputs arrive as few large-line DMAs (2-8KB/partition-line) on three
   engine queues, critical-path first (wk[t0], wq[t0], x chunks);
   output is stored bf16 and summed in f32 on the host.

Self-contained: no sibling imports; shapes hardcoded.
"""

import os
import numpy as np

import concourse.bass as bass
import concourse.mybir as mybir
import concourse.tile as tile
from concourse import bass_utils

B, N, C = 4, 2048, 1024
H = 16
D = 64
G = 2                 # head-groups (tensor-parallel split)
CL = C // G           # 512 local channels
HL = H // G           # 8 local heads
HP = HL // 2          # 4 head-pairs
KT = C // 128         # 8 k-tiles over c_in
NT = N // 128         # 16 m-tiles
NCH = N // 512        # 4 n-chunks
SCALE = D ** -0.5
LAG = 6               # units between score-exp and its AV consumption
NUNITS = NCH * HP * (NT // 2)   # 128

F32 = mybir.dt.float32
F32R = mybir.dt.float32r
BF16 = mybir.dt.bfloat16
FP8 = mybir.dt.float8e4

LAST_EXEC_TIME_NS = None


# ---------------------------------------------------------------- host utils

def _rne_f32r(x):
    """Round fp32 array to float32r (RNE to 11 explicit mantissa bits)."""
    u = np.ascontiguousarray(x, np.float32).view(np.uint32).astype(np.uint64)
    r = ((u + 0x800 + ((u >> 12) & 1)) >> 12) << 12
    return r.astype(np.uint32).view(np.float32)


def _softplus(x):
    x = x.astype(np.float32)
    return np.maximum(x, 0).astype(np.float32) + np.log1p(
        np.exp(-np.abs(x), dtype=np.float32), dtype=np.float32)


def _ntff_shim():
    """Register the axon NTFF profile hook if the image's antenv lacks it."""
    import sys, types
    try:
        from antenv.axon_hooks import get_axon_ntff_profile_hook  # noqa: F401
        return
    except ImportError:
        pass
    try:
        import antenv
        from trn_agent_boot.trn_boot import _ntff_profile_via_ctypes
        m = types.ModuleType("antenv.axon_hooks")
        m._hook = _ntff_profile_via_ctypes('/opt/axon/libaxon_pjrt.so')
        m.set_axon_ntff_profile_hook = lambda h: setattr(m, "_hook", h)
        m.get_axon_ntff_profile_hook = lambda: m._hook
        sys.modules["antenv.axon_hooks"] = m
        antenv.axon_hooks = m
    except Exception:
        pass


def _split_excess_waits(nc, limit=1):
    """walrus codegen allows few sync-waits per instruction; offload extras
    onto preceding NoOps on the same engine (program order preserves
    semantics)."""
    n_added = 0
    for fn in nc.m.functions:
        for blk in fn.blocks:
            new_insts = []
            for inst in blk.instructions:
                lim = limit
                si = inst.sync_info
                w = list(si.on_wait) if si and si.on_wait else []
                if len(w) > lim:
                    excess, keep = w[:-lim], w[-lim:]
                    for i in range(0, len(excess), limit):
                        chunk = excess[i:i + limit]
                        nop = mybir.InstNoOp(
                            name=f"{inst.name}-waitsplit-{i}", ins=[], outs=[])
                        nop.engine = inst.engine
                        nop.sync_info = mybir.SyncInfo(on_wait=chunk, on_update=[])
                        new_insts.append(nop)
                        n_added += 1
                    si.on_wait = keep
                new_insts.append(inst)
            blk.instructions[:] = new_insts
    return n_added


# ---------------------------------------------------------------- device code

def build_nc():
    nc = bass.Bass()
    xb_d = nc.declare_dram_parameter("xb", [128, NCH, KT, 512], BF16, isOutput=False)
    wq_d = nc.declare_dram_parameter("wq", [128, HP, KT, 128], BF16, isOutput=False)
    wk_d = nc.declare_dram_parameter("wk", [128, HP, KT, 128], BF16, isOutput=False)
    wv_d = nc.declare_dram_parameter("wv", [128, KT, CL], BF16, isOutput=False)
    pw_d = nc.declare_dram_parameter("pw", [128, HP, C], BF16, isOutput=False)
    y_d = nc.declare_dram_parameter("y", [128, NCH, 4, 2, 512], BF16, isOutput=True)

    with tile.TileContext(nc) as tc:
        with tc.tile_pool(name="persist", bufs=1) as pp, \
             tc.tile_pool(name="pmisc", bufs=2) as pm, \
             tc.tile_pool(name="py", bufs=4) as py, \
             tc.tile_pool(name="pao", bufs=3) as pao, \
             tc.tile_pool(name="ps2s", bufs=2, space="PSUM") as ps2s, \
             tc.tile_pool(name="ps2av", bufs=2, space="PSUM") as ps2av, \
             tc.tile_pool(name="ps2m", bufs=2, space="PSUM") as ps2m:
            q_sb = pp.tile([128, HP, N], BF16)          # Q^T
            k_sb = pp.tile([128, HP, N], BF16)          # K^T
            v_sb = pp.tile([128, NT, HL, 2 * D], BF16)  # V | 64 ones cols
            warm = pp.tile([128, 512], BF16)
            nc.vector.memset(warm[:], 0.0)
            nc.vector.memset(v_sb[:, :, :, D:2 * D], 1.0)
            pw_sb = pp.tile([128, HP, C], BF16)
            xb = pp.tile([128, NCH, KT, 512], BF16)
            wq = pp.tile([128, HP, KT, 128], BF16)
            wk = pp.tile([128, HP, KT, 128], BF16)
            wv = pp.tile([128, KT, CL], BF16)
            # probability ring: [side(2 heads of pair), m-tile slot, n]
            pr = pp.tile([128, 2, NT, 512], BF16)

            # DMAs: consumption order, large contiguous lines, split across
            # three engine queues (gpsimd/scalar idle during the head) so
            # descriptor issue and transfers overlap. The first score unit
            # needs only wk[t0], wq[t0], x[ch0] (1.5MB) — land those first.
            nc.sync.dma_start(wk[:, 0], wk_d[:, 0])
            nc.sync.dma_start(wq[:, 0], wq_d[:, 0])
            nc.sync.dma_start(xb[:, 0, 0:4], xb_d[:, 0, 0:4])
            nc.sync.dma_start(xb[:, 0, 4:8], xb_d[:, 0, 4:8])
            nc.sync.dma_start(xb[:, 1], xb_d[:, 1])
            nc.sync.dma_start(wk[:, 1], wk_d[:, 1])
            nc.sync.dma_start(xb[:, 2], xb_d[:, 2])
            nc.scalar.dma_start(wk[:, 2], wk_d[:, 2])
            nc.scalar.dma_start(wk[:, 3], wk_d[:, 3])
            nc.gpsimd.dma_start(wv[:], wv_d[:])
            nc.sync.dma_start(xb[:, 3], xb_d[:, 3])
            for t in range(1, HP):
                nc.scalar.dma_start(wq[:, t], wq_d[:, t])
            nc.gpsimd.dma_start(pw_sb[:], pw_d[:])

            def v_proj(mt, pool_tag=None):
                pool, tag = pool_tag or (ps2m, "m")
                ps = pool.tile([128, 512], F32, tag=tag, name=f"vp{mt}")
                for k in range(KT):
                    nc.tensor.matmul(
                        ps[:], xb[:, mt // 4, k, (mt % 4) * 128:(mt % 4 + 1) * 128],
                        wv[:, k, :], start=(k == 0), stop=(k == KT - 1))
                nc.vector.tensor_copy(
                    v_sb[:, mt, :, 0:D],
                    ps[:].rearrange("p (h d) -> p h d", h=HL))

            def qk_proj(dst, w, t, nchk, pool_tag=None):
                pool, tag = pool_tag or (ps2m, "m")
                ps = pool.tile([128, 512], F32, tag=tag, name=f"qk{t}_{nchk}")
                for k in range(KT):
                    nc.tensor.matmul(
                        ps[:], w[:, t, k, :], xb[:, nchk, k, :],
                        start=(k == 0), stop=(k == KT - 1))
                nc.vector.tensor_copy(
                    dst[:, t, nchk * 512:(nchk + 1) * 512], ps[:])

            ao_tiles = {}

            def outproj(nch, nt, cch, pool_tag=None):
                pool, tag = pool_tag or (ps2m, "m")
                yp = pool.tile([128, 512], F32, tag=tag, name=f"y{nch}_{nt}_{cch}")
                ao = ao_tiles[nch]
                for hp in range(HP):
                    nc.tensor.matmul(
                        yp[:], ao[:, hp, nt * 128:(nt + 1) * 128],
                        pw_sb[:, hp, cch * 512:(cch + 1) * 512],
                        start=(hp == 0), stop=(hp == HP - 1))
                y_sb = py.tile([128, 512], BF16, tag="y", name=f"ysb{nch}_{nt}_{cch}")
                nc.vector.tensor_copy(y_sb[:], yp[:])
                nc.sync.dma_start(y_d[:, nch, nt, cch, :], y_sb[:])

            av_tiles = {}

            def emit_scores_exp(nch, hp, g):
                nsl = slice(nch * 512, (nch + 1) * 512)
                for j in range(2):
                    mt = 2 * g + j
                    msl = slice(mt * 128, (mt + 1) * 128)
                    sj = ps2s.tile([128, 2, 512], F32, tag="s",
                                   name=f"s{nch}_{hp}_{mt}")
                    nc.tensor.matmul(
                        sj[:, 0], k_sb[0:64, hp, msl], q_sb[0:64, hp, nsl],
                        start=True, stop=True, tile_position=(0, 0))
                    nc.tensor.matmul(
                        sj[:, 1], k_sb[64:128, hp, msl], q_sb[64:128, hp, nsl],
                        start=True, stop=True, tile_position=(64, 0))
                    nc.scalar.activation(
                        pr[:, :, mt, :], sj[:],
                        mybir.ActivationFunctionType.Exp, scale=SCALE)

            def emit_av(nch, hp, g):
                if (nch, hp) not in av_tiles:
                    av_tiles[(nch, hp)] = (
                        ps2av.tile([128, 512], F32, tag="av", name=f"avA{nch}_{hp}"),
                        ps2av.tile([128, 512], F32, tag="av", name=f"avB{nch}_{hp}"))
                avA, avB = av_tiles[(nch, hp)]
                for j in range(2):
                    mt = 2 * g + j
                    nc.tensor.matmul(
                        avA[:], v_sb[:, mt, 2 * hp, :], pr[:, 0, mt, :],
                        start=(mt == 0), stop=(mt == NT - 1))
                    nc.tensor.matmul(
                        avB[:], v_sb[:, mt, 2 * hp + 1, :], pr[:, 1, mt, :],
                        start=(mt == 0), stop=(mt == NT - 1))

            tail_jobs = []
            tail_pools = [(ps2m, "m"), (ps2s, "s"), (ps2av, "av")]

            def norm_div(nch, hp, nd, side, base):
                rb = pm.tile([64, 512], F32, tag="rbi")
                nc.vector.reciprocal(rb[:], nd[D:2 * D, side, :])
                nc.vector.tensor_tensor(
                    ao_tiles[nch][base:base + 64, hp, :], nd[0:D, side, :],
                    rb[:], mybir.AluOpType.mult)

            def emit_normalize(nch, hp, at_unit, inline_outproj=False):
                avA, avB = av_tiles.pop((nch, hp))
                # copies first: frees BOTH av PSUM banks before the slow
                # reciprocals, so the next pair's AV matmuls can start.
                # A-copy rides the Scalar engine so the bank release never
                # queues behind a reciprocal on the DVE.
                nd = pm.tile([128, 2, 512], F32, tag="rb")
                nc.scalar.copy(nd[:, 0], avA[:])
                nc.vector.tensor_copy(nd[:, 1], avB[:])
                if inline_outproj:
                    # tail: nt-quarter split so the out-projection can start
                    # on the first quarter of the last reciprocal chain
                    ao_sb = ao_tiles[nch]
                    for nt in range(4):
                        nsl = slice(nt * 128, (nt + 1) * 128)
                        for side, base in ((0, 0), (1, 64)):
                            rb = pm.tile([64, 128], F32, tag="rbi")
                            nc.vector.reciprocal(rb[:], nd[D:2 * D, side, nsl])
                            nc.vector.tensor_tensor(
                                ao_sb[base:base + 64, hp, nsl],
                                nd[0:D, side, nsl], rb[:],
                                mybir.AluOpType.mult)
                        for ti, cch in enumerate((0, 1)):
                            outproj(nch, nt, cch,
                                    pool_tag=tail_pools[(2 * nt + ti) % 3])
                    return
                # spread the two ~4.3us reciprocal+mult chains into the next
                # units' filler slots: filler PSUM->SBUF casts (which feed
                # the next hp-block's scores / AV) then interleave between
                # them on the in-order DVE queue instead of queueing behind
                # a monolithic ~9us chain
                for i, (side, base) in enumerate(((0, 0), (1, 64))):
                    u = at_unit + 1 + i
                    job = (norm_div, (nch, hp, nd, side, base))
                    if u < NUNITS:
                        sched[u].append(job)
                    else:
                        tail_jobs.append(job)
                if hp == HP - 1:
                    # gate outproj ~9 units past the append: the scheduler's
                    # cost model underestimates the reciprocal chain ~6x, so
                    # an early outproj head-of-line-blocks the PE queue (and
                    # the resulting >3.4us PE idle re-throttles HAM)
                    i = 0
                    for nt in range(4):
                        for cch in range(2):
                            u = at_unit + 9 + 2 * i
                            if u < NUNITS:
                                sched[u].append((outproj, (nch, nt, cch)))
                            else:
                                tail_jobs.append((outproj, (nch, nt, cch)))
                            i += 1

            # ---------------- static just-in-time filler schedule
            sched = {u: [] for u in range(NUNITS)}
            jobs = []   # (need_by_unit, fn, args)
            for mt in range(NT):
                jobs.append((LAG + mt // 2 - 1, v_proj, (mt,)))
            # kp/qp get a 2-unit lead: their PSUM->SBUF cast can land behind
            # a ~4us reciprocal on the DVE queue
            for t in range(HP):
                for c in range(NCH):
                    if t == 0 and c == 0:
                        continue
                    jobs.append((8 * t + 2 * c - 3, qk_proj, (k_sb, wk, t, c)))
            for nchq in range(NCH):
                for t in range(HP):
                    if t == 0 and nchq == 0:
                        continue
                    jobs.append((32 * nchq + 8 * t - 3, qk_proj, (q_sb, wq, t, nchq)))
            CAP = 2
            for need_by, fn, args in sorted(jobs, key=lambda j: j[0]):
                u = max(0, min(need_by, NUNITS - 1))
                while u > 0 and len(sched[u]) >= CAP:
                    u -= 1
                sched[u].append((fn, args))

            # HAM warm-up: ~20 matmuls on a zeroed tile keep the PE activity
            # monitor at K=8/8 through the DMA-bound head so the first
            # projections run at 2.4GHz instead of 1.2
            wps = ps2m.tile([128, 512], F32, tag="m", name="warmps")
            for i in range(20):
                nc.tensor.matmul(wps[:], warm[:, 0:128], warm[:],
                                 start=True, stop=True)

            # warm-up: the minimum needed for unit (0,0,0)
            qk_proj(k_sb, wk, 0, 0)
            qk_proj(q_sb, wq, 0, 0)

            units = [(nch, hp, g) for nch in range(NCH)
                     for hp in range(HP) for g in range(NT // 2)]
            av_cursor = [0]

            def drain_av(upto, at_unit):
                while av_cursor[0] <= min(upto, NUNITS - 1):
                    lnch, lhp, lg = units[av_cursor[0]]
                    emit_av(lnch, lhp, lg)
                    if lg == NT // 2 - 1:
                        last = (av_cursor[0] == NUNITS - 1)
                        emit_normalize(lnch, lhp, at_unit,
                                       inline_outproj=last)
                    av_cursor[0] += 1

            for ui, (nch, hp, g) in enumerate(units):
                if hp == 0 and g == 0:
                    ao_tiles[nch] = pao.tile([128, HP, 512], BF16, tag="ao",
                                             name=f"ao{nch}")
                emit_scores_exp(nch, hp, g)
                # lag ramps down over the last units so the AV flush (and
                # the final normalize chain) starts before the last scores
                target = ui - LAG + max(0, min(ui - (NUNITS - LAG), LAG - 1))
                drain_av(target, ui)
                for fn, args in sched[ui]:
                    fn(*args)
            drain_av(NUNITS - 1, NUNITS)
            for ti, (fn, args) in enumerate(tail_jobs):
                if fn is outproj:
                    fn(*args, pool_tag=tail_pools[ti % 3])
                else:
                    fn(*args)
    return nc


# ---------------------------------------------------------------- entry point

def kernel(x, q_w, k_w, v_mu, v_rho, v_eps, proj_mu, proj_rho, proj_eps,
           pb_mu, pb_rho, pb_eps):
    global LAST_EXEC_TIME_NS
    _ntff_shim()
    import ml_dtypes

    x = np.asarray(x, np.float32)
    v_w = (np.asarray(v_mu, np.float32)
           + _softplus(np.asarray(v_rho)) * np.asarray(v_eps, np.float32))
    p_w = (np.asarray(proj_mu, np.float32)
           + _softplus(np.asarray(proj_rho)) * np.asarray(proj_eps, np.float32))
    p_b = (np.asarray(pb_mu, np.float32)
           + _softplus(np.asarray(pb_rho)) * np.asarray(pb_eps, np.float32))

    def wslice(w, g, t_major=False):
        # [128, KT, CL] bf16 layout of w[g*CL:(g+1)*CL, :].T
        # (t_major: [128, HP, KT, 128] with the head-pair tile outermost)
        wt = np.ascontiguousarray(np.asarray(w, np.float32)[g * CL:(g + 1) * CL, :].T)
        a = wt.reshape(KT, 128, CL).transpose(1, 0, 2)
        if t_major:
            a = a.reshape(128, KT, HP, 128).transpose(0, 2, 1, 3)
        return np.ascontiguousarray(a).astype(ml_dtypes.bfloat16)

    def pwslice(g):    # [128, HP, C] bf16 layout of p_w[:, g*CL:(g+1)*CL].T
        pt = np.ascontiguousarray(p_w[:, g * CL:(g + 1) * CL].T)
        return pt.reshape(HP, 128, C).transpose(1, 0, 2).astype(ml_dtypes.bfloat16)

    xts = []
    for b in range(B):
        xt = np.ascontiguousarray(x[b].T)          # [C, N]
        a = xt.reshape(KT, 128, NCH, 512)          # [k, p, nch, n']
        xts.append(np.ascontiguousarray(a.transpose(1, 2, 0, 3))
                   .astype(ml_dtypes.bfloat16))    # [p, nch, k, n']
    wqs = [wslice(q_w, g, t_major=True) for g in range(G)]
    wks = [wslice(k_w, g, t_major=True) for g in range(G)]
    wvs = [wslice(v_w, g) for g in range(G)]
    pws = [pwslice(g) for g in range(G)]

    in_maps = []
    for core in range(8):
        b, g = core // 2, core % 2
        in_maps.append({"xb": xts[b], "wq": wqs[g], "wk": wks[g],
                        "wv": wvs[g], "pw": pws[g]})

    nc = build_nc()
    _split_excess_waits(nc)
    res = bass_utils.run_bass_kernel_spmd(
        nc, in_maps, core_ids=list(range(8)),
        trace=bool(os.environ.get("BASS_TRACE")))
    LAST_EXEC_TIME_NS = res.exec_time_ns

    out = np.empty((B, N, C), np.float32)
    for b in range(B):
        yb = (np.asarray(res.results[2 * b]["y"]).astype(np.float32)
              + np.asarray(res.results[2 * b + 1]["y"]).astype(np.float32))
        # [p, nch, nt, cch, c'] -> [nch, nt, p, cch, c'] -> [N, C]
        out[b] = yb.transpose(1, 2, 0, 3, 4).reshape(N, C) + p_b
    return out
